# revision 1
# baseline (speedup 1.0000x reference)
"""Chamfer distance kernel for Trainium2 (8 NeuronCores, batch-parallel).

Strategy
--------
B=8 batches, one per core (SPMD: same program, per-core data).
Per core (N=M=8192, 3-D points):
  d[n,m] = |x1_n|^2 + |x2_m|^2 - 2 x1_n.x2_m  is computed fully inside
  PSUM by a single K=5 matmul with homogeneous coordinates:
    lhsT rows = [-2x, -2y, -2z, 1, n1],  rhs rows = [x', y', z', n2', 1]
  Two symmetric passes (rows = x1 points, then rows = x2 points) give the
  row-direction min+argmin for both outputs.  Per 128-row block the ACT
  engine moves PSUM->SBUF, the DVE does one reduce_min over [128, 8192]
  and one max_index (equality matcher, first occurrence == np.argmin
  tie-break) to extract the argmin.
Outputs come back as [128, 64] tiles (partition-major), the host
transpose-flattens them.
"""

import numpy as np

import concourse.bacc as bacc
import concourse.bass as bass
import concourse.mybir as mybir
from concourse import tile
from concourse.bass_utils import run_bass_kernel_spmd

F32 = mybir.dt.float32
I32 = mybir.dt.int32
U32 = mybir.dt.uint32

_PROGRAM_CACHE = {}


def _emit_pass(nc, lhsU, rhsU, base, dtile, itile, rowbuf_pool, psum_pool,
               scratch_pool, n_pts, m_pts, mm_dt):
    """One direction: for each 128-row block of lhs points, min+argmin over
    all m_pts columns.  The lhsT role slice is lhsU[base:base+5] =
    [-2x, -2y, -2z, 1, n]; the rhs role slice is rhsU[base:base+5] =
    [x, y, z, n, 1] (matmul requires equal operand base partitions)."""
    n_blocks = n_pts // 128
    n_groups = m_pts // 2048

    for nb in range(n_blocks):
        rowbuf = rowbuf_pool.tile([128, m_pts], F32, tag="rowbuf")
        lhs_ap = lhsU[base:base + 5, nb * 128:(nb + 1) * 128]
        if mm_dt is not F32:
            lhs_ap = lhs_ap.bitcast(mm_dt)
        for g in range(n_groups):
            psum = psum_pool.tile([128, 2048], F32, tag="psum")
            for q in range(4):
                rhs_ap = rhsU[base:base + 5, (g * 4 + q) * 512:(g * 4 + q + 1) * 512]
                if mm_dt is not F32:
                    rhs_ap = rhs_ap.bitcast(mm_dt)
                nc.tensor.matmul(
                    psum[:, q * 512:(q + 1) * 512],
                    lhs_ap,
                    rhs_ap,
                    start=True, stop=True,
                )
            nc.scalar.activation(
                rowbuf[:, g * 2048:(g + 1) * 2048], psum[:],
                mybir.ActivationFunctionType.Copy,
            )
        # row min -> dist column
        nc.vector.tensor_reduce(
            dtile[:, nb:nb + 1], rowbuf[:],
            axis=mybir.AxisListType.X, op=mybir.AluOpType.min,
        )
        # match the min value back to its first position
        q8 = scratch_pool.tile([128, 8], F32, tag="q8")
        ix = scratch_pool.tile([128, 8], U32, tag="ix")
        nc.vector.tensor_copy(q8[:], dtile[:, nb:nb + 1].broadcast_to((128, 8)))
        nc.vector.max_index(ix[:], q8[:], rowbuf[:])
        nc.vector.tensor_copy(itile[:, nb:nb + 1], ix[:, 0:1])


def _build_program(n_pts=8192, m_pts=8192, n_cores=8, mm_dtype="f32", repeat=1):
    key = (n_pts, m_pts, n_cores, mm_dtype, repeat)
    if key in _PROGRAM_CACHE:
        return _PROGRAM_CACHE[key]

    mm_dt = {"f32": F32, "f32r": mybir.dt.float32r}[mm_dtype]

    nc = bacc.Bacc("TRN2", target_bir_lowering=False, debug=False,
                   num_devices=n_cores)
    # rows [x, y, z, ones]; the ones row seeds the homogeneous-coordinate rows
    c1 = nc.dram_tensor("c1", [4, n_pts], F32, kind="ExternalInput")
    c2 = nc.dram_tensor("c2", [4, m_pts], F32, kind="ExternalInput")
    d1 = nc.dram_tensor("d1", [128, n_pts // 128], F32, kind="ExternalOutput")
    i1 = nc.dram_tensor("i1", [128, n_pts // 128], I32, kind="ExternalOutput")
    d2 = nc.dram_tensor("d2", [128, m_pts // 128], F32, kind="ExternalOutput")
    i2 = nc.dram_tensor("i2", [128, m_pts // 128], I32, kind="ExternalOutput")

    with tile.TileContext(nc) as tc:
        with tc.tile_pool(name="persist", bufs=1) as persist, \
             tc.tile_pool(name="finals", bufs=1) as finals:
            # Combined tiles.  Matmul needs BOTH operands at the same base
            # partition (0/32/64), so:
            #   U1: lhsT-form (A = [-2x,-2y,-2z, 1, n1]) at rows 0-4,
            #       rhs-form  (B = [x, y, z, n1, 1])     at rows 32-36
            #   U2: rhs-form  (B = [x', y', z', n2', 1]) at rows 0-4,
            #       lhsT-form (A = [-2x',-2y',-2z', 1, n2]) at rows 32-36
            # Pass A pairs U1[0:5] x U2[0:5]; pass B pairs U2[32:37] x U1[32:37].
            # One 32KB column range per tensor covers both forms.
            U1 = persist.tile([37, n_pts], F32, tag="U1")
            U2 = persist.tile([37, m_pts], F32, tag="U2")
            # ones column for the norm matmuls; [35, 1] so both base-0 and
            # base-32 slices exist (matmul operands must share their base)
            ones_col = persist.tile([35, 1], F32, tag="ones_col")
            nc.vector.memset(ones_col[:], 1.0)

            d1t = finals.tile([128, n_pts // 128], F32, tag="d1t")
            i1t = finals.tile([128, n_pts // 128], I32, tag="i1t")
            d2t = finals.tile([128, m_pts // 128], F32, tag="d2t")
            i2t = finals.tile([128, m_pts // 128], I32, tag="i2t")

            # ---- prep both tensors ----
            with tc.tile_pool(name="prep", bufs=1) as prep, \
                 tc.tile_pool(name="preppsum", bufs=2, space="PSUM") as ppsum:
                # a = base row of the A-form, b = base row of the B-form.
                # Engine ops keep all APs at one base (partition quadrant
                # rule); DMA moves rows across bases.
                for U, c, npts, a, b in ((U1, c1, n_pts, 0, 32),
                                         (U2, c2, m_pts, 32, 0)):
                    nc.sync.dma_start(U[b:b + 3, :], c.ap()[0:3, :])   # B coords
                    nc.sync.dma_start(U[a + 3:a + 4, :], c.ap()[3:4, :])  # A ones
                    nc.sync.dma_start(U[b + 4:b + 5, :], c.ap()[3:4, :])  # B ones
                    sq = prep.tile([35, npts], F32, tag="sq")
                    nrow = prep.tile([1, npts], F32, tag="nrow")
                    nc.scalar.activation(sq[b:b + 3, :], U[b:b + 3, :],
                                         mybir.ActivationFunctionType.Square)
                    for cchunk in range(npts // 512):
                        ps = ppsum.tile([1, 512], F32, tag="ps")
                        nc.tensor.matmul(ps[:], ones_col[b:b + 3, :],
                                         sq[b:b + 3, cchunk * 512:(cchunk + 1) * 512],
                                         start=True, stop=True)
                        nc.scalar.activation(
                            nrow[:, cchunk * 512:(cchunk + 1) * 512],
                            ps[:], mybir.ActivationFunctionType.Copy)
                    nc.sync.dma_start(U[b + 3:b + 4, :], nrow[:])  # B n-row
                    nc.sync.dma_start(U[a + 4:a + 5, :], nrow[:])  # A n-row
                    # A coords = -2 * B coords: scale in place (same base),
                    # then DMA into the A rows
                    nc.vector.tensor_scalar(
                        out=sq[b:b + 3, :], in0=U[b:b + 3, :],
                        scalar1=-2.0, scalar2=None,
                        op0=mybir.AluOpType.mult)
                    nc.sync.dma_start(U[a:a + 3, :], sq[b:b + 3, :])

            # ---- main passes ----
            with tc.tile_pool(name="rowbuf", bufs=2) as rowbuf_pool, \
                 tc.tile_pool(name="mainpsum", bufs=2, space="PSUM") as psum_pool, \
                 tc.tile_pool(name="scratch", bufs=2) as scratch_pool:
                for _ in range(repeat):
                    _emit_pass(nc, U1, U2, 0, d1t, i1t, rowbuf_pool, psum_pool,
                               scratch_pool, n_pts, m_pts, mm_dt)
                    _emit_pass(nc, U2, U1, 32, d2t, i2t, rowbuf_pool, psum_pool,
                               scratch_pool, m_pts, n_pts, mm_dt)

            # clamp tiny negative rounding like the reference's max(d, 0)
            nc.scalar.activation(d1t[:], d1t[:], mybir.ActivationFunctionType.Relu)
            nc.scalar.activation(d2t[:], d2t[:], mybir.ActivationFunctionType.Relu)
            nc.sync.dma_start(d1.ap(), d1t[:])
            nc.sync.dma_start(i1.ap(), i1t[:])
            nc.sync.dma_start(d2.ap(), d2t[:])
            nc.sync.dma_start(i2.ap(), i2t[:])

    nc.compile()
    _PROGRAM_CACHE[key] = nc
    return nc


def kernel(xyz1: np.ndarray, xyz2: np.ndarray, mm_dtype: str = "f32",
           repeat: int = 1, _return_results_only: bool = False):
    xyz1 = np.asarray(xyz1, dtype=np.float32)
    xyz2 = np.asarray(xyz2, dtype=np.float32)
    B, N, _ = xyz1.shape
    _, M, _ = xyz2.shape
    assert B == 8 and N == 8192 and M == 8192, (B, N, M)

    nc = _build_program(N, M, B, mm_dtype, repeat)

    ones_n = np.ones((1, N), np.float32)
    ones_m = np.ones((1, M), np.float32)
    in_maps = [
        {"c1": np.concatenate([np.ascontiguousarray(xyz1[b].T), ones_n]),
         "c2": np.concatenate([np.ascontiguousarray(xyz2[b].T), ones_m])}
        for b in range(B)
    ]
    res = run_bass_kernel_spmd(nc, in_maps, list(range(B)))

    dist1 = np.empty((B, N), np.float32)
    dist2 = np.empty((B, M), np.float32)
    idx1 = np.empty((B, N), np.int32)
    idx2 = np.empty((B, M), np.int32)
    for b in range(B):
        r = res.results[b]
        dist1[b] = np.asarray(r["d1"]).T.reshape(-1)
        idx1[b] = np.asarray(r["i1"]).T.reshape(-1)
        dist2[b] = np.asarray(r["d2"]).T.reshape(-1)
        idx2[b] = np.asarray(r["i2"]).T.reshape(-1)
    return dist1, dist2, idx1, idx2



# revision 2
# speedup vs baseline: 1.0678x; 1.0678x over previous
"""Chamfer distance kernel for Trainium2 (8 NeuronCores, batch-parallel) — v2.

B=8 batches, one per core (SPMD). Per core (N=M=8192, 3-D points), two
direction passes (rows = x1 points, then rows = x2 points); 64 row-blocks
of 128 points each; per block, 4 column-groups of 2048.

Key ideas vs the K=5 fp32 baseline (3.4 ms of mostly fp32-matmul + DVE):

1. bf16 triple-split matmul (K=24): each coordinate/norm row is split into
   hi/mid/lo bf16 parts; the 6 product pairs that matter (hh, hm, mh, hl,
   lh, mm) plus the norm rows are laid out as 24 stationary/moving row
   pairs, so ONE bf16 matmul per 512-column chunk produces d to ~1e-7
   absolute accuracy (1 argmin flip in 65536 rows on the reference data) at
   ~4.5x the fp32 matmul rate.

2. Fused suffix-min scan: groups are processed g=3..0. One DVE
   tensor_scalar(op0=min vs previous suffix-min, op1=min reduce,
   accum_out) per group reads PSUM once and yields
   s_g = min(group_g min, s_{g+1}) with zero extra ops. s_0 is the row
   (block) min -> dist output.

3. Exact-indicator strips: ACT writes t = Exp((s_g - d)*2^67) per group
   into a bf16 strip. 2^67 is a power of two so s_g*2^67 is exact: the
   strip is exactly 1.0 where d attains the suffix min and 0.0 elsewhere
   (any nonzero f32 gap scales to < -128 before Exp). Groups biased by
   the SUFFIX min mean: groups before the winning group have no 1.0 at
   all, so the FIRST 1.0 in the 8192-wide strip is the global argmin,
   with np.argmin's first-occurrence tie semantics preserved exactly.

4. ONE bf16 max_index per block (query = constant 1.0) extracts the
   argmin; max_index on bf16 runs ~4x faster than on fp32.

Host does the bf16 splitting (prep) and the final relu/slicing.
"""

import numpy as np
import ml_dtypes

import concourse.bacc as bacc
import concourse.mybir as mybir
from concourse import tile
from concourse.bass_utils import run_bass_kernel_spmd

F32 = mybir.dt.float32
BF16 = mybir.dt.bfloat16
U32 = mybir.dt.uint32
AF = mybir.ActivationFunctionType
ALU = mybir.AluOpType

BF = ml_dtypes.bfloat16
SCALE = float(2.0 ** 67)

_PROGRAM_CACHE = {}


def _build_program(n_pts=8192, n_cores=8, repeat=1):
    key = (n_pts, n_cores, repeat)
    if key in _PROGRAM_CACHE:
        return _PROGRAM_CACHE[key]

    NB = n_pts // 128          # row blocks per direction
    NG = 4                     # column groups per block
    GW = n_pts // NG           # group width (2048)

    nc = bacc.Bacc("TRN2", target_bir_lowering=False, debug=False,
                   num_devices=n_cores)
    uu = nc.dram_tensor("uu", [96, n_pts], BF16, kind="ExternalInput")
    s1o = nc.dram_tensor("s1", [128, NB * NG], F32, kind="ExternalOutput")
    x1o = nc.dram_tensor("x1", [128, NB * 8], U32, kind="ExternalOutput")
    s2o = nc.dram_tensor("s2", [128, NB * NG], F32, kind="ExternalOutput")
    x2o = nc.dram_tensor("x2", [128, NB * 8], U32, kind="ExternalOutput")

    with tile.TileContext(nc) as tc:
        with tc.tile_pool(name="persist", bufs=1) as persist:
            # rows 0-23: A-form (stationary side), rows 32-55: B-form
            # (moving side). Pass A pairs U1[0:24] x U2[0:24]; pass B pairs
            # U2[32:56] x U1[32:56] (matmul operands share a base quadrant).
            U1 = persist.tile([56, n_pts], BF16, tag="U1")
            U2 = persist.tile([56, n_pts], BF16, tag="U2")
            q1 = persist.tile([128, 8], BF16, tag="q1")
            big = persist.tile([128, 1], F32, tag="big")
            sacc1 = persist.tile([128, NB * NG], F32, tag="sacc1")
            sacc2 = persist.tile([128, NB * NG], F32, tag="sacc2")
            xacc1 = persist.tile([128, NB * 8], U32, tag="xacc1")
            xacc2 = persist.tile([128, NB * 8], U32, tag="xacc2")

            nc.vector.memset(q1[:], 1.0)
            nc.vector.memset(big[:], 3.4e38)
            nc.sync.dma_start(U1[0:24, :], uu.ap()[0:24, :])
            nc.sync.dma_start(U1[32:56, :], uu.ap()[24:48, :])
            nc.sync.dma_start(U2[0:24, :], uu.ap()[72:96, :])
            nc.sync.dma_start(U2[32:56, :], uu.ap()[48:72, :])

            dirs = ((U1, U2, 0, sacc1, xacc1),
                    (U2, U1, 32, sacc2, xacc2))

            with tc.tile_pool(name="psum", bufs=2, space="PSUM") as pspool, \
                 tc.tile_pool(name="tb", bufs=4) as tbpool:
                for _ in range(repeat):
                    # The two direction passes are interleaved block by block
                    # so each pass's DVE work hides the other's ACT/PSUM
                    # dependency chain; each block's max_index is emitted one
                    # interleave step late for the same reason.
                    pend = [None, None]

                    def emit_block(di, nb):
                        lhsU, rhsU, base, sacc, xacc = dirs[di]
                        tb = tbpool.tile([128, n_pts], BF16, tag="tb")
                        lhs = lhsU[base:base + 24, nb * 128:(nb + 1) * 128]
                        for g in range(NG - 1, -1, -1):
                            ps = pspool.tile([128, GW], F32, tag="ps")
                            for q in range(GW // 512):
                                c0 = g * GW + q * 512
                                nc.tensor.matmul(
                                    ps[:, q * 512:(q + 1) * 512],
                                    lhs,
                                    rhsU[base:base + 24, c0:c0 + 512],
                                    start=True, stop=True)
                            c = nb * NG + g
                            sprev = (big[:, 0:1] if g == NG - 1
                                     else sacc[:, c + 1:c + 2])
                            # suffix min: s_g = min(min(group), s_{g+1}); the
                            # elementwise out is junk (overwritten by the Exp)
                            nc.vector.tensor_scalar(
                                out=tb[:, g * GW:(g + 1) * GW],
                                in0=ps[:], scalar1=sprev, scalar2=None,
                                op0=ALU.min, op1=ALU.min,
                                accum_out=sacc[:, c:c + 1])
                            # psum holds d*2^67 (A-rows pre-scaled on the
                            # host), so the suffix min IS the Exp bias
                            nc.scalar.activation(
                                tb[:, g * GW:(g + 1) * GW], ps[:],
                                AF.Exp, bias=sacc[:, c:c + 1], scale=-1.0)
                        pend[di] = (tb, nb)

                    def emit_maxidx(di):
                        if pend[di] is None:
                            return
                        xacc = dirs[di][4]
                        ptb, pnb = pend[di]
                        nc.vector.max_index(
                            xacc[:, pnb * 8:(pnb + 1) * 8], q1[:], ptb[:])
                        pend[di] = None

                    for nb in range(NB):
                        emit_block(0, nb)
                        emit_maxidx(1)
                        emit_block(1, nb)
                        emit_maxidx(0)
                    emit_maxidx(1)

            nc.sync.dma_start(s1o.ap(), sacc1[:])
            nc.sync.dma_start(x1o.ap(), xacc1[:])
            nc.sync.dma_start(s2o.ap(), sacc2[:])
            nc.sync.dma_start(x2o.ap(), xacc2[:])

    nc.compile()
    _PROGRAM_CACHE[key] = nc
    return nc


def _split3(v):
    """bf16 triple split: v ~= h + m + l with each part bf16-representable
    (returned as f32 numpy arrays)."""
    h = v.astype(BF).astype(np.float32)
    r = (v - h).astype(np.float32)
    m = r.astype(BF).astype(np.float32)
    l = (r - m).astype(BF).astype(np.float32)
    return h, m, l


def _forms(xyz):
    """[N,3] f32 -> (A, B) [24, N] bf16 homogeneous triple-split forms.

    Row pairing (A row i) * (B row i), summed by the PE:
      0-2  (-2x_h, x'_h)   3-5  (-2x_h, x'_m)   6-8  (-2x_m, x'_h)
      9-11 (-2x_h, x'_l)  12-14 (-2x_l, x'_h)  15-17 (-2x_m, x'_m)
      18-20 (1, n'_{h,m,l})  21-23 (n_{h,m,l}, 1)
    = n + n' - 2(hh+hm+mh+hl+lh+mm) ~= squared distance.
    """
    x = np.ascontiguousarray(xyz.T).astype(np.float32)      # [3, N]
    n = (x * x).sum(0, dtype=np.float32)[None, :]           # [1, N]
    s = (-2.0 * x).astype(np.float32)
    sh, sm, sl = _split3(s)
    xh, xm, xl = _split3(x)
    nh, nm, nl = _split3(n)
    ones = np.ones_like(n)
    A = (np.concatenate([sh, sh, sm, sh, sl, sm, ones, ones, ones,
                         nh, nm, nl]) * SCALE).astype(BF)
    Bf = np.concatenate([xh, xm, xh, xl, xh, xm, nh, nm, nl,
                         ones, ones, ones]).astype(BF)
    return A, Bf


def kernel(xyz1: np.ndarray, xyz2: np.ndarray, repeat: int = 1):
    xyz1 = np.asarray(xyz1, dtype=np.float32)
    xyz2 = np.asarray(xyz2, dtype=np.float32)
    B, N, _ = xyz1.shape
    M = xyz2.shape[1]
    assert B == 8 and N == 8192 and M == 8192, (B, N, M)

    nc = _build_program(N, B, repeat)

    in_maps = []
    for b in range(B):
        A1, B1 = _forms(xyz1[b])
        A2, B2 = _forms(xyz2[b])
        in_maps.append({"uu": np.concatenate([A1, B1, A2, B2])})
    res = run_bass_kernel_spmd(nc, in_maps, list(range(B)))

    NB = N // 128
    dist1 = np.empty((B, N), np.float32)
    dist2 = np.empty((B, M), np.float32)
    idx1 = np.empty((B, N), np.int32)
    idx2 = np.empty((B, M), np.int32)
    for b in range(B):
        r = res.results[b]
        for s_name, x_name, dist, idx in (("s1", "x1", dist1, idx1),
                                          ("s2", "x2", dist2, idx2)):
            s = np.asarray(r[s_name])          # [128, NB*4]
            xi = np.asarray(r[x_name])         # [128, NB*8] u32
            vmin = s[:, 0::4] * (1.0 / SCALE)  # s_0 per block -> [128, NB]
            ix = xi[:, 0::8].astype(np.int64)  # slot 0 -> [128, NB]
            dist[b] = np.maximum(vmin, 0.0).T.reshape(-1)
            idx[b] = ix.T.reshape(-1).astype(np.int32)
    return dist1, dist2, idx1, idx2
